# revision 1
# baseline (speedup 1.0000x reference)
"""Trainium2 Bass kernel for nn_FCGFAutoencoder (segment_max -> 3-layer MLP decoder).

Strategy (data-parallel over segments, per sharding hint):
  - batch_ids are sorted, so the host finds the 65 segment boundaries with
    searchsorted and repacks features into a [B, cap, C] array padded with
    -3e38 (max-identity).  Each of the 8 cores gets 8 whole segments.
  - Layout per (segment, core): partition p holds rows [p*L, (p+1)*L) of the
    segment -> each DMA is a single huge contiguous read (~2MB per quarter
    segment), which is required to reach HBM roofline (~358 GB/s/core).
  - On-device: per quarter-segment tile [128, (L/4)*32], a single strided
    reduce_max over the row axis gives [128, 32]; 3 tensor_max combines,
    a PE transpose to [32, 128], and a final reduce_max yield gT[:, s].
  - Decoder (replicated tiny MLP): h1T = relu(W1^T gT + b1), h2T = relu(W2^T
    h1T + b2), out = h2^T W3 + b3, all on PE/ACT/DVE, single [8, 3072] store.
  - Host concatenates the 8 per-core [8, 3072] outputs -> [64, 3, 1024].
"""

import os
import sys
import types

sys.path.insert(0, "/opt/trn_rl_repo")

import numpy as np


def _ensure_axon_hooks():
    """Some images lack antenv.axon_hooks; bass_utils imports it when
    trace=True under axon. Install a shim that lazily wires the real
    ctypes-based NTFF hook from trn_agent_boot if present, else degrades
    to no-trace instead of crashing."""
    try:
        import antenv.axon_hooks  # noqa: F401

        return
    except ImportError:
        pass
    try:
        import antenv
    except ImportError:
        return
    mod = types.ModuleType("antenv.axon_hooks")
    _hook = [None]

    def set_axon_ntff_profile_hook(h):
        _hook[0] = h

    def get_axon_ntff_profile_hook():
        if _hook[0] is None:
            try:
                from trn_agent_boot.trn_boot import _ntff_profile_via_ctypes

                _hook[0] = _ntff_profile_via_ctypes("/opt/axon/libaxon_pjrt.so")
            except Exception:
                return None
        return _hook[0]

    mod.set_axon_ntff_profile_hook = set_axon_ntff_profile_hook
    mod.get_axon_ntff_profile_hook = get_axon_ntff_profile_hook
    sys.modules["antenv.axon_hooks"] = mod
    antenv.axon_hooks = mod

N = 4_194_304
C = 32
B = 64
NUM_POINTS = 1024
NCORES = 8
SPC = B // NCORES  # segments per core
P = 128
J = 4  # DMA chunks per segment
NEG = -3.0e38
H1, H2, OUT_D = 256, 512, 3 * NUM_POINTS
K1, K2, NT = H1 // P, H2 // P, OUT_D // 512

LAST_RESULTS = None

_build_cache = {}


def _build(cap):
    if cap in _build_cache:
        return _build_cache[cap]

    import concourse.bacc as bacc
    import concourse.tile as tile
    from concourse import mybir
    from concourse.masks import make_identity
    from contextlib import ExitStack

    L = cap // P  # rows per partition per segment
    LQ = L // J  # rows per partition per DMA chunk
    F = LQ * C  # free elems per chunk tile

    f32 = mybir.dt.float32
    AX = mybir.AxisListType.X
    nc = bacc.Bacc("TRN2", target_bir_lowering=False)

    feats = nc.dram_tensor("feats", [SPC * cap, C], f32, kind="ExternalInput")
    w1 = nc.dram_tensor("w1", [C, H1], f32, kind="ExternalInput")
    b1t = nc.dram_tensor("b1t", [P, K1], f32, kind="ExternalInput")
    w2 = nc.dram_tensor("w2", [H1, H2], f32, kind="ExternalInput")
    b2t = nc.dram_tensor("b2t", [P, K2], f32, kind="ExternalInput")
    w3 = nc.dram_tensor("w3", [H2, OUT_D], f32, kind="ExternalInput")
    b3r = nc.dram_tensor("b3r", [SPC, OUT_D], f32, kind="ExternalInput")
    out = nc.dram_tensor("out", [SPC, OUT_D], f32, kind="ExternalOutput")

    # rows: s*cap + p*L + j*LQ + i  ->  [s, j, p, (i c)]
    fview = feats[:].rearrange("(s p j i) c -> s j p (i c)", s=SPC, p=P, j=J)

    with ExitStack() as ctx:
        tc = ctx.enter_context(tile.TileContext(nc))
        consts = ctx.enter_context(tc.tile_pool(name="consts", bufs=1))
        fpool = ctx.enter_context(tc.tile_pool(name="feat", bufs=6))
        outp = ctx.enter_context(tc.tile_pool(name="outp", bufs=2))
        redp = ctx.enter_context(tc.tile_pool(name="red", bufs=2 * J))
        ptr = ctx.enter_context(tc.tile_pool(name="ptr", bufs=2, space="PSUM"))
        pmm = ctx.enter_context(tc.tile_pool(name="pmm", bufs=2, space="PSUM"))
        pout = ctx.enter_context(tc.tile_pool(name="pout", bufs=2, space="PSUM"))

        ident = consts.tile([P, P], f32)
        make_identity(nc, ident)

        # weight/bias loads on the SP HWDGE ring; feature streaming runs on
        # the Act ring (so the ACT observer copies below share its engine).
        # biases first: the Act-ring observer copies below wait on these
        # lanes, and SP-ring DMAs are FIFO -- queueing them behind 6.5MB of
        # weights would stall the feature stream start by ~20us.
        b1_sb = consts.tile([P, K1], f32)
        nc.sync.dma_start(out=b1_sb, in_=b1t[:])
        b2_sb = consts.tile([P, K2], f32)
        nc.sync.dma_start(out=b2_sb, in_=b2t[:])
        HS = SPC // 2  # segments per decoder half
        b3_sb = []
        for h in range(2):
            bh = consts.tile([HS, OUT_D], f32, tag=f"b3h{h}")
            nc.sync.dma_start(out=bh, in_=b3r[h * HS : (h + 1) * HS])
            b3_sb.append(bh)
        w1_sb = consts.tile([C, H1], f32)
        nc.sync.dma_start(out=w1_sb, in_=w1[:])
        w2_sb = consts.tile([P, K1, H2], f32)
        nc.sync.dma_start(out=w2_sb, in_=w2[:].rearrange("(k p) n -> p k n", p=P))
        w3_sb = consts.tile([P, K2, OUT_D], f32)
        nc.sync.dma_start(out=w3_sb, in_=w3[:].rearrange("(k p) n -> p k n", p=P))

        # Observer copies: each engine may carry only ONE semaphore wait per
        # instruction, so advance ACT's and DVE's vector clocks over the
        # bias-load DMA lanes early; the decoder relu/add ops then need only
        # their PE wait.
        obs = consts.tile([1, 16], f32)
        nc.vector.tensor_copy(out=obs[0:1, 3:4], in_=b3_sb[0][0:1, 0:1])
        nc.vector.tensor_copy(out=obs[0:1, 4:5], in_=b3_sb[1][0:1, 0:1])

        # PE (Matmult/LDW) supports only ONE sync wait per instruction, so a
        # matmul whose inputs come from two unobserved semaphores fails to
        # compile. Prime PE with throwaway single-wait transposes so it has
        # observed the identity (Pool) and each weight-DMA lane before the
        # real matmuls. Each gets its own PSUM slot (slot reuse would add a
        # second, PE-release wait); the pool closes before the others open.
        with tc.tile_pool(name="prime", bufs=1, space="PSUM") as primep:
            prime_srcs = (
                ident[:, 0:C],
                w1_sb[:, 0:C],
                w2_sb[:, 0, 0:C],
                w3_sb[:, 0, 0:C],
            )
            pp = primep.tile([C, P], f32, tag="prime")
            for src in prime_srcs:
                kk = src.shape[0]
                nc.tensor.transpose(
                    out=pp[0:C, 0:kk], in_=src, identity=ident[0:kk, 0:kk]
                )

        gT = consts.tile([C, SPC], f32)
        segobs = consts.tile([1, SPC], f32)

        RB = 8  # row-blocks kept per chunk; small levels are overhead-bound

        def chunk_tree(eng, ft, scr, rj):
            # contiguous tree max over the row axis: pairs (i, c) with
            # (i + n/2, c); ping-pong between ft and scratch. Stops at RB
            # blocks (tail levels are fixed-overhead-dominated); rj is
            # [P, RB*C] and the cross-chunk combine finishes the job.
            cur, nxt = ft, ft
            n = LQ
            while n > 2 * RB:
                if n % 2 == 1:
                    eng.tensor_max(
                        cur[:, 0:C], cur[:, 0:C], cur[:, (n - 1) * C : n * C]
                    )
                    n -= 1
                half = n // 2
                eng.tensor_max(
                    nxt[:, 0 : half * C],
                    cur[:, 0 : half * C],
                    cur[:, half * C : n * C],
                )
                cur, nxt = nxt, cur
                n = half
            while n % RB:
                eng.tensor_max(cur[:, 0:C], cur[:, 0:C], cur[:, (n - 1) * C : n * C])
                n -= 1
            eng.tensor_max(
                rj[:, :], cur[:, 0 : (n // 2) * C], cur[:, (n // 2) * C : n * C]
            )

        def decode_half(h):
            # decoder for segments [h*HS, (h+1)*HS): runs while the other
            # half is still streaming, so only the last half is tail time.
            cols = slice(h * HS, (h + 1) * HS)
            # empty segments: reference maps -inf -> 0; padding is -3e38, so
            # mask = (g > -1e37) in {0,1}; g * mask zeroes empties exactly.
            mask = consts.tile([C, HS], f32, tag=f"mask{h}")
            gfix = consts.tile([C, HS], f32, tag=f"gfix{h}")
            nc.vector.tensor_scalar(
                out=mask[:, :],
                in0=gT[:, cols],
                scalar1=-1.0e37,
                scalar2=None,
                op0=mybir.AluOpType.is_gt,
            )
            nc.vector.tensor_mul(gfix[:, :], gT[:, cols], mask[:, :])

            # h1T[m] = relu(W1[:, m]^T @ g + b1[m])   [128, HS] per chunk m
            h1_sb = consts.tile([P, K1, HS], f32, tag=f"h1{h}")
            for m in range(K1):
                pm = pmm.tile([P, HS], f32, tag="pm")
                nc.tensor.matmul(
                    pm[:, :],
                    w1_sb[:, m * P : (m + 1) * P],
                    gfix[:, :],
                    start=True,
                    stop=True,
                )
                nc.scalar.activation(
                    out=h1_sb[:, m, :],
                    in_=pm[:, :],
                    func=mybir.ActivationFunctionType.Relu,
                    bias=b1_sb[:, m : m + 1],
                    scale=1.0,
                )

            # h2T[m] = relu(sum_k W2[k, :, m]^T @ h1T[k] + b2[m])
            h2_sb = consts.tile([P, K2, HS], f32, tag=f"h2{h}")
            for m in range(K2):
                pm = pmm.tile([P, HS], f32, tag="pm")
                for k in range(K1):
                    nc.tensor.matmul(
                        pm[:, :],
                        w2_sb[:, k, m * P : (m + 1) * P],
                        h1_sb[:, k, :],
                        start=(k == 0),
                        stop=(k == K1 - 1),
                    )
                nc.scalar.activation(
                    out=h2_sb[:, m, :],
                    in_=pm[:, :],
                    func=mybir.ActivationFunctionType.Relu,
                    bias=b2_sb[:, m : m + 1],
                    scale=1.0,
                )

            # out[:, n] = sum_k h2T[k]^T @ W3[k, :, n] + b3[:, n]
            # streamed per 512-column chunk through a small rotating tile
            for n in range(NT):
                po = pout.tile([HS, 512], f32, tag="po")
                for k in range(K2):
                    nc.tensor.matmul(
                        po[:, :],
                        h2_sb[:, k, :],
                        w3_sb[:, k, n * 512 : (n + 1) * 512],
                        start=(k == 0),
                        stop=(k == K2 - 1),
                    )
                ob = outp.tile([HS, 512], f32, tag="ob")
                nc.vector.tensor_add(
                    ob[:, :],
                    po[:, :],
                    b3_sb[h][:, n * 512 : (n + 1) * 512],
                )
                # SWDGE store: DMASW lanes unused by the feature stream.
                nc.gpsimd.dma_start(
                    out=out[h * HS : (h + 1) * HS, n * 512 : (n + 1) * 512],
                    in_=ob[:, :],
                )

        for s in range(SPC):
            reds = []
            for j in range(J):
                ft = fpool.tile([P, F], f32, tag="ft")
                nc.scalar.dma_start(out=ft, in_=fview[s, j])
                rj = redp.tile([P, RB * C], f32, tag="rj")
                chunk_tree(nc.vector, ft, None, rj)
                reds.append(rj)
                if j == 0:
                    # ACT observers: advance Act's DVE and Pool clocks past
                    # the tree reads of chunks 0-1 (one per engine), covering
                    # the slot releases the NEXT segment's reuse-DMAs
                    # (issued from Act) depend on -- they then wait only on
                    # their own DMA lane, and the pipeline never drains at
                    # segment boundaries.
                    nc.scalar.copy(
                        out=segobs[0:1, s : s + 1], in_=rj[0:1, 0:1]
                    )
            stride = 1
            while stride < J:
                for a in range(0, J, 2 * stride):
                    nc.vector.tensor_max(
                        reds[a][:, :], reds[a][:, :], reds[a + stride][:, :]
                    )
                stride *= 2
            if s == 0:
                # ACT observers for the bias lanes, emitted after segment
                # 0's feature DMAs so they never delay stream start; they
                # only need to precede the decoder relus.
                nc.scalar.copy(out=obs[0:1, 0:1], in_=b1_sb[0:1, 0:1])
                nc.scalar.copy(out=obs[0:1, 1:2], in_=b2_sb[0:1, 0:1])
                nc.scalar.copy(out=obs[0:1, 2:3], in_=b3_sb[0][0:1, 0:1])
            rs = reds[0]
            n = RB
            while n > 1:
                half = n // 2
                nc.vector.tensor_max(
                    rs[:, 0 : half * C],
                    rs[:, 0 : half * C],
                    rs[:, half * C : n * C],
                )
                n = half
            pt = ptr.tile([C, P], f32, tag="pt")
            nc.tensor.transpose(
                out=pt[:, :], in_=rs[:, 0:C], identity=ident[:, :]
            )
            nc.vector.reduce_max(out=gT[:, s : s + 1], in_=pt[:, :], axis=AX)
            if s == SPC // 2 - 1:
                decode_half(0)

        decode_half(1)
    nc.compile()
    _build_cache[cap] = nc
    return nc


def kernel(**inputs):
    global LAST_RESULTS
    features = np.ascontiguousarray(np.asarray(inputs["features"], dtype=np.float32))
    batch_ids = np.asarray(inputs["batch_ids"])
    W1 = np.ascontiguousarray(np.asarray(inputs["W1"], dtype=np.float32))
    b1 = np.asarray(inputs["b1"], dtype=np.float32)
    W2 = np.ascontiguousarray(np.asarray(inputs["W2"], dtype=np.float32))
    b2 = np.asarray(inputs["b2"], dtype=np.float32)
    W3 = np.ascontiguousarray(np.asarray(inputs["W3"], dtype=np.float32))
    b3 = np.asarray(inputs["b3"], dtype=np.float32)

    bounds = np.searchsorted(batch_ids, np.arange(B + 1), side="left")
    seg_len = np.diff(bounds)
    maxlen = max(1, int(seg_len.max()))
    L = -(-maxlen // P)  # ceil
    L = -(-L // J) * J  # round up to multiple of J
    L = max(L, 64)  # keep LQ >= 16 so the tree structure holds
    cap = L * P

    packed = np.empty((B, cap, C), np.float32)
    for b in range(B):
        lo, hi = int(bounds[b]), int(bounds[b + 1])
        n = hi - lo
        packed[b, :n] = features[lo:hi]
        packed[b, n:] = NEG

    b1t = np.ascontiguousarray(b1.reshape(K1, P).T)
    b2t = np.ascontiguousarray(b2.reshape(K2, P).T)
    b3r = np.ascontiguousarray(np.broadcast_to(b3, (SPC, OUT_D)))

    nc = _build(cap)

    in_maps = []
    for d in range(NCORES):
        in_maps.append(
            {
                "feats": packed[d * SPC : (d + 1) * SPC].reshape(SPC * cap, C),
                "w1": W1,
                "b1t": b1t,
                "w2": W2,
                "b2t": b2t,
                "w3": W3,
                "b3r": b3r,
            }
        )

    _ensure_axon_hooks()
    from concourse.bass_utils import run_bass_kernel_spmd

    core_ids = list(range(NCORES))
    try:
        res = run_bass_kernel_spmd(nc, in_maps, core_ids=core_ids)
    except Exception:
        if os.environ.get("BASS_TRACE") and not os.environ.get("BASS_NEVER_TRACE"):
            # trace post-processing can fail in restricted containers;
            # retry without tracing so the numeric result still lands.
            os.environ["BASS_NEVER_TRACE"] = "1"
            try:
                res = run_bass_kernel_spmd(nc, in_maps, core_ids=core_ids)
            finally:
                os.environ.pop("BASS_NEVER_TRACE", None)
        else:
            raise
    LAST_RESULTS = res

    full = np.concatenate([r["out"] for r in res.results], axis=0)
    return full.reshape(B, 3, NUM_POINTS)



# revision 9
# speedup vs baseline: 1.7888x; 1.7888x over previous
"""Trainium2 Bass kernel for nn_FCGFAutoencoder (segment_max -> 3-layer MLP decoder).

Strategy (data-parallel over segments, per sharding hint):
  - batch_ids are sorted, so the host finds the 65 segment boundaries with
    searchsorted and repacks features into a [B, cap, C] fp16 array padded
    with -65504 (fp16 max-identity).  Each of the 8 cores gets 8 whole
    segments.  fp16 halves HBM traffic (memory-bound kernel) and puts the
    DVE tensor_tensor max tree in its 2x_1P perf mode.
  - Layout per (segment, core): partition p holds rows [p*L, (p+1)*L) of the
    segment -> each DMA is a single huge contiguous read per partition.
  - On-device: per quarter-segment tile [128, (L/4)*32], a pairwise tree of
    tensor_max ops over the row axis gives [128, 32]; 3 tensor_max combines,
    a PE transpose to [32, 128], and a final reduce_max yield gT[:, s].
  - Decoder (replicated tiny MLP, W2/W3/h1/h2 in bf16 for 2x PE rate and
    half the weight DMA): h1T = relu(W1^T gT + b1), h2T = relu(W2^T h1T +
    b2), out = h2^T W3 + b3.  Segments 0-5 decode mid-stream; 6-7 at the
    end so the decode tail is only ~1/4 of the decoder.
  - Host concatenates the 8 per-core [8, 3072] outputs -> [64, 3, 1024].
"""

import os
import sys
import types

sys.path.insert(0, "/opt/trn_rl_repo")

import numpy as np
import ml_dtypes


def _ensure_axon_hooks():
    """Some images lack antenv.axon_hooks; bass_utils imports it when
    trace=True under axon. Install a shim that lazily wires the real
    ctypes-based NTFF hook from trn_agent_boot if present, else degrades
    to no-trace instead of crashing."""
    try:
        import antenv.axon_hooks  # noqa: F401

        return
    except ImportError:
        pass
    try:
        import antenv
    except ImportError:
        return
    mod = types.ModuleType("antenv.axon_hooks")
    _hook = [None]

    def set_axon_ntff_profile_hook(h):
        _hook[0] = h

    def get_axon_ntff_profile_hook():
        if _hook[0] is None:
            try:
                from trn_agent_boot.trn_boot import _ntff_profile_via_ctypes

                _hook[0] = _ntff_profile_via_ctypes("/opt/axon/libaxon_pjrt.so")
            except Exception:
                return None
        return _hook[0]

    mod.set_axon_ntff_profile_hook = set_axon_ntff_profile_hook
    mod.get_axon_ntff_profile_hook = get_axon_ntff_profile_hook
    sys.modules["antenv.axon_hooks"] = mod
    antenv.axon_hooks = mod

N = 4_194_304
C = 32
B = 64
NUM_POINTS = 1024
NCORES = 8
SPC = B // NCORES  # segments per core
P = 128
J = 4  # DMA chunks per segment
NEG = -65504.0  # fp16 lowest normal; max-identity for the padded tail
H1, H2, OUT_D = 256, 512, 3 * NUM_POINTS
K1, K2, NT = H1 // P, H2 // P, OUT_D // 512
SPLIT = 6  # segments [0, SPLIT) decode mid-stream, [SPLIT, SPC) at the end

LAST_RESULTS = None

_build_cache = {}


def _build(cap):
    if cap in _build_cache:
        return _build_cache[cap]

    import concourse.bacc as bacc
    import concourse.tile as tile
    from concourse import mybir
    from concourse.masks import make_identity
    from contextlib import ExitStack

    L = cap // P  # rows per partition per segment
    LQ = L // J  # rows per partition per DMA chunk
    F = LQ * C  # free elems per chunk tile

    f32 = mybir.dt.float32
    f16 = mybir.dt.float16
    bf16 = mybir.dt.bfloat16
    AX = mybir.AxisListType.X
    nc = bacc.Bacc("TRN2", target_bir_lowering=False)

    feats = nc.dram_tensor("feats", [SPC * cap, C], f16, kind="ExternalInput")
    w1 = nc.dram_tensor("w1", [C, H1], f32, kind="ExternalInput")
    b1t = nc.dram_tensor("b1t", [P, K1], f32, kind="ExternalInput")
    w2 = nc.dram_tensor("w2", [H1, H2], bf16, kind="ExternalInput")
    b2t = nc.dram_tensor("b2t", [P, K2], f32, kind="ExternalInput")
    w3 = nc.dram_tensor("w3", [H2, OUT_D], bf16, kind="ExternalInput")
    b3r = nc.dram_tensor("b3r", [SPC, OUT_D], f32, kind="ExternalInput")
    out = nc.dram_tensor("out", [SPC, OUT_D], f32, kind="ExternalOutput")

    # rows: s*cap + p*L + j*LQ + i  ->  [s, j, p, (i c)]
    fview = feats[:].rearrange("(s p j i) c -> s j p (i c)", s=SPC, p=P, j=J)

    with ExitStack() as ctx:
        tc = ctx.enter_context(tile.TileContext(nc))
        consts = ctx.enter_context(tc.tile_pool(name="consts", bufs=1))
        fpool = ctx.enter_context(tc.tile_pool(name="feat", bufs=8))
        outp = ctx.enter_context(tc.tile_pool(name="outp", bufs=2))
        redp = ctx.enter_context(tc.tile_pool(name="red", bufs=2 * J))
        ptr = ctx.enter_context(tc.tile_pool(name="ptr", bufs=2, space="PSUM"))
        pmm = ctx.enter_context(tc.tile_pool(name="pmm", bufs=2, space="PSUM"))
        pout = ctx.enter_context(tc.tile_pool(name="pout", bufs=2, space="PSUM"))

        ident = consts.tile([P, P], f32)
        make_identity(nc, ident)
        identb = consts.tile([P, P], bf16, tag="identb")
        make_identity(nc, identb)

        # weight/bias loads on the SP HWDGE ring; feature streaming runs on
        # the Act ring (so the ACT observer copies below share its engine).
        # biases first: the Act-ring observer copies below wait on these
        # lanes, and SP-ring DMAs are FIFO -- queueing them behind MBs of
        # weights would stall the feature stream start.
        b1_sb = consts.tile([P, K1], f32)
        nc.sync.dma_start(out=b1_sb, in_=b1t[:])
        b2_sb = consts.tile([P, K2], f32)
        nc.sync.dma_start(out=b2_sb, in_=b2t[:])
        b3_sb = []
        for h, (lo, hi) in enumerate(((0, SPLIT), (SPLIT, SPC))):
            bh = consts.tile([hi - lo, OUT_D], f32, tag=f"b3h{h}")
            nc.sync.dma_start(out=bh, in_=b3r[lo:hi])
            b3_sb.append(bh)
        w1_sb = consts.tile([C, H1], f32)
        nc.sync.dma_start(out=w1_sb, in_=w1[:])
        w2_sb = consts.tile([P, K1, H2], bf16)
        nc.sync.dma_start(out=w2_sb, in_=w2[:].rearrange("(k p) n -> p k n", p=P))
        w3_sb = consts.tile([P, K2, OUT_D], bf16)
        nc.sync.dma_start(out=w3_sb, in_=w3[:].rearrange("(k p) n -> p k n", p=P))

        # Observer copies: each engine may carry only ONE semaphore wait per
        # instruction, so advance ACT's and DVE's vector clocks over the
        # bias-load DMA lanes early; the decoder relu/add ops then need only
        # their PE wait.
        obs = consts.tile([1, 16], f32)
        nc.vector.tensor_copy(out=obs[0:1, 3:4], in_=b3_sb[0][0:1, 0:1])
        nc.vector.tensor_copy(out=obs[0:1, 4:5], in_=b3_sb[1][0:1, 0:1])

        # PE (Matmult/LDW) supports only ONE sync wait per instruction, so a
        # matmul whose inputs come from two unobserved semaphores fails to
        # compile. Prime PE with throwaway single-wait transposes so it has
        # observed both identities (Pool) and each weight-DMA lane before the
        # real matmuls. Each gets its own PSUM slot (slot reuse would add a
        # second, PE-release wait); the pool closes before the others open.
        with tc.tile_pool(name="prime", bufs=1, space="PSUM") as primep:
            pp = primep.tile([C, P], f32, tag="prime")
            nc.tensor.transpose(
                out=pp[0:C, 0:P], in_=ident[:, 0:C], identity=ident[:, :]
            )
            nc.tensor.transpose(
                out=pp[0:C, 0:C], in_=w1_sb[:, 0:C], identity=ident[0:C, 0:C]
            )
            ppb = primep.tile([C, P], bf16, tag="primeb")
            nc.tensor.transpose(
                out=ppb[0:C, 0:P], in_=identb[:, 0:C], identity=identb[:, :]
            )
            nc.tensor.transpose(
                out=ppb[0:C, 0:P], in_=w2_sb[:, 0, 0:C], identity=identb[:, :]
            )
            nc.tensor.transpose(
                out=ppb[0:C, 0:P], in_=w3_sb[:, 0, 0:C], identity=identb[:, :]
            )

        gT = consts.tile([C, SPC], f32)
        segobs = consts.tile([1, SPC], f32)

        RB = 8  # row-blocks kept per chunk; small levels are overhead-bound

        def chunk_tree(eng, ft, rj):
            # contiguous tree max over the row axis: pairs (i, c) with
            # (i + n/2, c); ping-pong within ft. Stops at RB blocks (tail
            # levels are fixed-overhead-dominated); rj is [P, RB*C] and the
            # cross-chunk combine finishes the job.
            cur, nxt = ft, ft
            n = LQ
            while n > 2 * RB:
                if n % 2 == 1:
                    eng.tensor_max(
                        cur[:, 0:C], cur[:, 0:C], cur[:, (n - 1) * C : n * C]
                    )
                    n -= 1
                half = n // 2
                eng.tensor_max(
                    nxt[:, 0 : half * C],
                    cur[:, 0 : half * C],
                    cur[:, half * C : n * C],
                )
                cur, nxt = nxt, cur
                n = half
            while n % RB:
                eng.tensor_max(cur[:, 0:C], cur[:, 0:C], cur[:, (n - 1) * C : n * C])
                n -= 1
            eng.tensor_max(
                rj[:, :], cur[:, 0 : (n // 2) * C], cur[:, (n // 2) * C : n * C]
            )

        def decode_range(h, lo, hi):
            # decoder for segments [lo, hi): all but the last range run while
            # later segments are still streaming, so only the last is tail.
            HS = hi - lo
            cols = slice(lo, hi)
            # empty segments: reference maps -inf -> 0; padding is -65504,
            # so mask = (g > -60000) in {0,1}; g * mask zeroes empties.
            mask = consts.tile([C, HS], f32, tag=f"mask{h}")
            gfix = consts.tile([C, HS], f32, tag=f"gfix{h}")
            nc.vector.tensor_scalar(
                out=mask[:, :],
                in0=gT[:, cols],
                scalar1=-60000.0,
                scalar2=None,
                op0=mybir.AluOpType.is_gt,
            )
            nc.vector.tensor_mul(gfix[:, :], gT[:, cols], mask[:, :])

            # h1T[m] = relu(W1[:, m]^T @ g + b1[m])   [128, HS] per chunk m
            h1_sb = consts.tile([P, K1, HS], bf16, tag=f"h1{h}")
            for m in range(K1):
                pm_full = pmm.tile([P, SPLIT], f32, tag="pm")
                pm = pm_full[:, 0:HS]
                nc.tensor.matmul(
                    pm[:, :],
                    w1_sb[:, m * P : (m + 1) * P],
                    gfix[:, :],
                    start=True,
                    stop=True,
                )
                nc.scalar.activation(
                    out=h1_sb[:, m, :],
                    in_=pm[:, :],
                    func=mybir.ActivationFunctionType.Relu,
                    bias=b1_sb[:, m : m + 1],
                    scale=1.0,
                )

            # h2T[m] = relu(sum_k W2[k, :, m]^T @ h1T[k] + b2[m])
            h2_sb = consts.tile([P, K2, HS], bf16, tag=f"h2{h}")
            for m in range(K2):
                pm_full = pmm.tile([P, SPLIT], f32, tag="pm")
                pm = pm_full[:, 0:HS]
                for k in range(K1):
                    nc.tensor.matmul(
                        pm[:, :],
                        w2_sb[:, k, m * P : (m + 1) * P],
                        h1_sb[:, k, :],
                        start=(k == 0),
                        stop=(k == K1 - 1),
                    )
                nc.scalar.activation(
                    out=h2_sb[:, m, :],
                    in_=pm[:, :],
                    func=mybir.ActivationFunctionType.Relu,
                    bias=b2_sb[:, m : m + 1],
                    scale=1.0,
                )

            # out[:, n] = sum_k h2T[k]^T @ W3[k, :, n] + b3[:, n]
            # streamed per 512-column chunk through a small rotating tile
            for n in range(NT):
                po_full = pout.tile([SPLIT, 512], f32, tag="po")
                po = po_full[0:HS, :]
                for k in range(K2):
                    nc.tensor.matmul(
                        po[:, :],
                        h2_sb[:, k, :],
                        w3_sb[:, k, n * 512 : (n + 1) * 512],
                        start=(k == 0),
                        stop=(k == K2 - 1),
                    )
                ob = outp.tile([HS, 512], f32, tag=f"ob{h}")
                nc.vector.tensor_add(
                    ob[:, :],
                    po[:, :],
                    b3_sb[h][:, n * 512 : (n + 1) * 512],
                )
                # SWDGE store: DMASW lanes unused by the feature stream.
                nc.gpsimd.dma_start(
                    out=out[lo:hi, n * 512 : (n + 1) * 512],
                    in_=ob[:, :],
                )

        for s in range(SPC):
            reds = []
            for j in range(J):
                ft = fpool.tile([P, F], f16, tag="ft")
                nc.scalar.dma_start(out=ft, in_=fview[s, j])
                rj = redp.tile([P, RB * C], f16, tag="rj")
                chunk_tree(nc.vector, ft, rj)
                reds.append(rj)
                if j == 0:
                    # ACT observers: advance Act's DVE clock past the tree
                    # reads of chunk 0, covering the slot releases the NEXT
                    # segment's reuse-DMAs (issued from Act) depend on --
                    # they then wait only on their own DMA lane, and the
                    # pipeline never drains at segment boundaries.
                    nc.scalar.copy(
                        out=segobs[0:1, s : s + 1], in_=rj[0:1, 0:1]
                    )
            stride = 1
            while stride < J:
                for a in range(0, J, 2 * stride):
                    nc.vector.tensor_max(
                        reds[a][:, :], reds[a][:, :], reds[a + stride][:, :]
                    )
                stride *= 2
            if s == 0:
                # ACT observers for the bias lanes, emitted after segment
                # 0's feature DMAs so they never delay stream start; they
                # only need to precede the decoder relus.
                nc.scalar.copy(out=obs[0:1, 0:1], in_=b1_sb[0:1, 0:1])
                nc.scalar.copy(out=obs[0:1, 1:2], in_=b2_sb[0:1, 0:1])
                nc.scalar.copy(out=obs[0:1, 2:3], in_=b3_sb[0][0:1, 0:1])
            rs = reds[0]
            n = RB
            while n > 2:
                half = n // 2
                nc.vector.tensor_max(
                    rs[:, 0 : half * C],
                    rs[:, 0 : half * C],
                    rs[:, half * C : n * C],
                )
                n = half
            # final combine converts fp16 -> fp32 for the PE transpose
            rs32 = redp.tile([P, C], f32, tag="rs32")
            nc.vector.tensor_max(rs32[:, :], rs[:, 0:C], rs[:, C : 2 * C])
            pt = ptr.tile([C, P], f32, tag="pt")
            nc.tensor.transpose(
                out=pt[:, :], in_=rs32[:, :], identity=ident[:, :]
            )
            nc.vector.reduce_max(out=gT[:, s : s + 1], in_=pt[:, :], axis=AX)
            if s == SPLIT - 1:
                decode_range(0, 0, SPLIT)

        decode_range(1, SPLIT, SPC)
    nc.compile()
    _build_cache[cap] = nc
    return nc


def kernel(**inputs):
    global LAST_RESULTS
    features = np.asarray(inputs["features"], dtype=np.float32)
    batch_ids = np.asarray(inputs["batch_ids"])
    W1 = np.ascontiguousarray(np.asarray(inputs["W1"], dtype=np.float32))
    b1 = np.asarray(inputs["b1"], dtype=np.float32)
    W2 = np.ascontiguousarray(
        np.asarray(inputs["W2"], dtype=np.float32).astype(ml_dtypes.bfloat16)
    )
    b2 = np.asarray(inputs["b2"], dtype=np.float32)
    W3 = np.ascontiguousarray(
        np.asarray(inputs["W3"], dtype=np.float32).astype(ml_dtypes.bfloat16)
    )
    b3 = np.asarray(inputs["b3"], dtype=np.float32)

    bounds = np.searchsorted(batch_ids, np.arange(B + 1), side="left")
    seg_len = np.diff(bounds)
    maxlen = max(1, int(seg_len.max()))
    L = -(-maxlen // P)  # ceil
    L = -(-L // J) * J  # round up to multiple of J
    L = max(L, 64)  # keep LQ >= 16 so the tree structure holds
    cap = L * P

    packed = np.empty((B, cap, C), np.float16)
    feats16 = features.astype(np.float16)
    for b in range(B):
        lo, hi = int(bounds[b]), int(bounds[b + 1])
        n = hi - lo
        packed[b, :n] = feats16[lo:hi]
        packed[b, n:] = NEG

    b1t = np.ascontiguousarray(b1.reshape(K1, P).T)
    b2t = np.ascontiguousarray(b2.reshape(K2, P).T)
    b3r = np.ascontiguousarray(np.broadcast_to(b3, (SPC, OUT_D)))

    nc = _build(cap)

    in_maps = []
    for d in range(NCORES):
        in_maps.append(
            {
                "feats": packed[d * SPC : (d + 1) * SPC].reshape(SPC * cap, C),
                "w1": W1,
                "b1t": b1t,
                "w2": W2,
                "b2t": b2t,
                "w3": W3,
                "b3r": b3r,
            }
        )

    _ensure_axon_hooks()
    from concourse.bass_utils import run_bass_kernel_spmd

    core_ids = list(range(NCORES))
    try:
        res = run_bass_kernel_spmd(nc, in_maps, core_ids=core_ids)
    except Exception:
        if os.environ.get("BASS_TRACE") and not os.environ.get("BASS_NEVER_TRACE"):
            # trace post-processing can fail in restricted containers;
            # retry without tracing so the numeric result still lands.
            os.environ["BASS_NEVER_TRACE"] = "1"
            try:
                res = run_bass_kernel_spmd(nc, in_maps, core_ids=core_ids)
            finally:
                os.environ.pop("BASS_NEVER_TRACE", None)
        else:
            raise
    LAST_RESULTS = res

    full = np.concatenate([r["out"] for r in res.results], axis=0)
    return full.reshape(B, 3, NUM_POINTS)
